# revision 17
# baseline (speedup 1.0000x reference)
"""Routed quantized MoE eval kernel for 8 Trainium2 NeuronCores.

Strategy (expert-parallel, per sharding hint):
- Core c owns expert e=c: quantized expert weights are dequantized
  (scale-folded) + transposed on the host at shard-prep time; the
  matmuls, router, top-2 softmax, SwiGLU activations and combine all
  run on device.
- Shared MLP is sharded along DF_S: core c computes rows
  [256c, 256c+256) of the shared gate/up and the matching columns of
  the down projection, giving a partial shared output.
- Every core computes the full router (fp32 matmuls - top-2 selection
  is tie-sensitive), forms its own expert's combine column
  ca[:, e] * alpha[e] and (1 - sum_e ca*alpha), scales its expert
  output and shared partial, and the per-token sum across all 8 cores
  is taken by chunked ReduceScatter collectives that overlap compute.
- Big matmuls run as float32r (2 cyc/row, ~1.5e-4 rel err), router in
  true float32.

Output identity used:
  mixed = (1 - sum_e ca_e*alpha_e) * shared + sum_e ca_e*alpha_e * eo_e
where shared = sum over cores of shared partials, so each core's
contribution is (1-s)*shared_partial_c + ca_c*alpha_c*eo_c.
"""

import numpy as np
from contextlib import ExitStack

import concourse.bass as bass
import concourse.tile as tile
from concourse import bacc, mybir
from concourse.bass_utils import run_bass_kernel_spmd

NCORES = 8
B, S, D = 2, 1024, 1024
T = B * S                      # 2048 tokens
DF_E, DF_S, E = 512, 2048, 8
FS = DF_S // NCORES            # 256 shared-ffn rows per core
CH = 4                         # token chunks
CT = T // CH                   # 512 tokens per chunk
TT = CT // 128                 # 4 token tiles per chunk
KD = D // 128                  # 8 k-tiles over hidden dim
KF = DF_E // 128               # 4 k-tiles over expert ffn dim
KS = FS // 128                 # 2 k-tiles over shared ffn shard
ND = D // 512                  # 2 output column slices

import os

FR = mybir.dt.float32r
F16 = mybir.dt.float16
F32 = mybir.dt.float32
DT_MODE = os.environ.get("MOE_DT", "f32r")
DT_MM = {"f32r": FR, "f16": F16}[DT_MODE]
NP_MM = {"f32r": np.float32, "f16": np.float16}[DT_MODE]
ACTF = mybir.ActivationFunctionType
ALU = mybir.AluOpType

_CACHE = {}


def _build():
    nc = bacc.Bacc(
        "TRN2", target_bir_lowering=False, debug=False, num_devices=NCORES
    )

    xT = nc.dram_tensor("xT", [D, T], DT_MM, kind="ExternalInput").ap()
    xTf = nc.dram_tensor("xTf", [D, T], F32, kind="ExternalInput").ap()
    rwT = nc.dram_tensor("rwT", [D, E], F32, kind="ExternalInput").ap()
    gqT = nc.dram_tensor("gqT", [D, DF_E], DT_MM, kind="ExternalInput").ap()
    uqT = nc.dram_tensor("uqT", [D, DF_E], DT_MM, kind="ExternalInput").ap()
    dqT = nc.dram_tensor("dqT", [DF_E, D], DT_MM, kind="ExternalInput").ap()
    wgT = nc.dram_tensor("wgT", [D, FS], DT_MM, kind="ExternalInput").ap()
    wuT = nc.dram_tensor("wuT", [D, FS], DT_MM, kind="ExternalInput").ap()
    wdT = nc.dram_tensor("wdT", [FS, D], DT_MM, kind="ExternalInput").ap()
    # aux[:, 0:8] = alpha broadcast, aux[:, 8:16] = onehot(expert) broadcast
    aux = nc.dram_tensor("aux", [128, 2 * E], F32, kind="ExternalInput").ap()
    OUT = nc.dram_tensor(
        "OUT", [CH, CT // NCORES, D], F32, kind="ExternalOutput"
    ).ap()

    with ExitStack() as ctx:
        tc = ctx.enter_context(tile.TileContext(nc))
        wres = ctx.enter_context(tc.tile_pool(name="wres", bufs=1))
        xs = ctx.enter_context(tc.tile_pool(name="xs", bufs=2))
        xfp = ctx.enter_context(tc.tile_pool(name="xfp", bufs=1))
        hp = ctx.enter_context(tc.tile_pool(name="hp", bufs=2))
        work = ctx.enter_context(tc.tile_pool(name="work", bufs=2))
        rt = ctx.enter_context(tc.tile_pool(name="rt", bufs=2))
        ps_gu = ctx.enter_context(tc.tile_pool(name="ps_gu", bufs=3, space="PSUM"))
        ps_dn = ctx.enter_context(tc.tile_pool(name="ps_dn", bufs=3, space="PSUM"))
        ps_r = ctx.enter_context(tc.tile_pool(name="ps_r", bufs=2, space="PSUM"))
        dram = ctx.enter_context(tc.tile_pool(name="dram", bufs=1, space="DRAM"))

        # ---- resident weights ----------------------------------------
        def load_rows(src, rows, cols, name):
            tiles = []
            r = src.rearrange("(k p) n -> k p n", p=128)
            for k in range(rows // 128):
                t = wres.tile([128, cols], src.dtype, tag=f"{name}{k}")
                nc.sync.dma_start(t[:], r[k])
                tiles.append(t)
            return tiles

        # router weights + aux first (tiny, unblock router matmuls)
        from concourse.masks import make_identity

        ident = wres.tile([128, 128], F32, tag="ident")
        make_identity(nc, ident[:])
        rw = load_rows(rwT, D, E, "rw")
        aux_sb = wres.tile([128, 2 * E], F32, tag="aux")
        nc.sync.dma_start(aux_sb[:], aux[:])
        alpha_bc = aux_sb[:, 0:E]
        sel_bc = aux_sb[:, E : 2 * E]

        xTr = xT.rearrange("(k p) t -> k p t", p=128)
        xTfr = xTf.rearrange("(k p) t -> k p t", p=128)

        def load_x(c):
            xt, xf_t = [], []
            for k in range(KD):
                tf = xfp.tile([128, CT], F32, tag=f"xf{k}")
                nc.sync.dma_start(tf[:], xTfr[k, :, c * CT : (c + 1) * CT])
                xf_t.append(tf)
                t = xs.tile([128, CT], DT_MM, tag=f"xt{k}")
                nc.sync.dma_start(t[:], xTr[k, :, c * CT : (c + 1) * CT])
                xt.append(t)
            return xt, xf_t

        x_pre = load_x(0)
        gq = load_rows(gqT, D, DF_E, "gq")
        uq = load_rows(uqT, D, DF_E, "uq")
        dq = load_rows(dqT, DF_E, D, "dq")
        wg = load_rows(wgT, D, FS, "wg")
        wu = load_rows(wuT, D, FS, "wu")
        wd = load_rows(wdT, FS, D, "wd")

        for c in range(CH):
            xt, xf_t = x_pre
            if c + 1 < CH:
                x_pre = load_x(c + 1)

            # ---- router + combine weights ----------------------------
            ps_lt = ps_r.tile([E, CT], F32, tag="psr")
            for k in range(KD):
                nc.tensor.matmul(
                    ps_lt[:],
                    rw[k][:],
                    xf_t[k][:],
                    start=(k == 0),
                    stop=(k == KD - 1),
                )
            Lt = rt.tile([E, CT], F32, tag="Lt")
            nc.vector.tensor_copy(Lt[:], ps_lt[:])
            # transpose the 4 token-tile blocks into one [128, TT*E] tile
            ps_l = ps_r.tile([128, TT * E], F32, tag="psr")
            for j in range(TT):
                nc.tensor.transpose(
                    ps_l[:, j * E : (j + 1) * E],
                    Lt[:, j * 128 : (j + 1) * 128],
                    ident[0:E, 0:E],
                )
            L = rt.tile([128, TT * E], F32, tag="L")
            nc.vector.tensor_copy(L[:], ps_l[:])
            L3 = L[:].rearrange("p (j e) -> p j e", e=E)

            def bc(t):  # [128, TT] -> [128, TT, E] free-axis broadcast
                return t[:, :, None].broadcast_to([128, TT, E])

            m1 = rt.tile([128, TT], F32, tag="m1")
            nc.vector.tensor_reduce(m1[:], L3, mybir.AxisListType.X, ALU.max)
            mask1 = rt.tile([128, TT * E], F32, tag="mask1")
            mask1_3 = mask1[:].rearrange("p (j e) -> p j e", e=E)
            nc.vector.tensor_tensor(mask1_3, L3, bc(m1), op=ALU.is_ge)
            L2 = rt.tile([128, TT * E], F32, tag="L2")
            nc.vector.scalar_tensor_tensor(
                L2[:], mask1[:], -1e30, L[:], ALU.mult, ALU.add
            )
            L2_3 = L2[:].rearrange("p (j e) -> p j e", e=E)
            m2 = rt.tile([128, TT], F32, tag="m2")
            nc.vector.tensor_reduce(m2[:], L2_3, mybir.AxisListType.X, ALU.max)
            mask2 = rt.tile([128, TT * E], F32, tag="mask2")
            mask2_3 = mask2[:].rearrange("p (j e) -> p j e", e=E)
            nc.vector.tensor_tensor(mask2_3, L2_3, bc(m2), op=ALU.is_ge)
            # softmax over {m1, m2}: w1 = sigmoid(m1 - m2), w2 = 1 - w1
            dlt = rt.tile([128, TT], F32, tag="dlt")
            nc.vector.tensor_sub(dlt[:], m1[:], m2[:])
            w1 = rt.tile([128, TT], F32, tag="w1")
            nc.scalar.activation(w1[:], dlt[:], ACTF.Sigmoid)
            w2 = rt.tile([128, TT], F32, tag="w2")
            nc.vector.tensor_scalar(w2[:], w1[:], -1.0, 1.0, ALU.mult, ALU.add)
            caw = rt.tile([128, TT * E], F32, tag="caw")
            caw3 = caw[:].rearrange("p (j e) -> p j e", e=E)
            nc.vector.tensor_tensor(caw3, mask2_3, bc(w2), op=ALU.mult)
            t1 = rt.tile([128, TT * E], F32, tag="t1")
            t1_3 = t1[:].rearrange("p (j e) -> p j e", e=E)
            nc.vector.tensor_tensor(t1_3, mask1_3, bc(w1), op=ALU.mult)
            nc.vector.tensor_add(caw[:], caw[:], t1[:])
            # scale by alpha (broadcast over token-tiles) and reduce
            ca_a = rt.tile([128, TT * E], F32, tag="ca_a")
            ca_a3 = ca_a[:].rearrange("p (j e) -> p j e", e=E)
            alpha3 = alpha_bc[:, None, :].broadcast_to([128, TT, E])
            nc.vector.tensor_tensor(ca_a3, caw3, alpha3, op=ALU.mult)
            s = rt.tile([128, TT], F32, tag="s")
            nc.vector.tensor_reduce(s[:], ca_a3, mybir.AxisListType.X, ALU.add)
            om_all = rt.tile([128, TT], F32, tag=f"om")
            nc.vector.tensor_scalar(om_all[:], s[:], -1.0, 1.0, ALU.mult, ALU.add)
            selm = rt.tile([128, TT * E], F32, tag="selm")
            selm3 = selm[:].rearrange("p (j e) -> p j e", e=E)
            sel3 = sel_bc[:, None, :].broadcast_to([128, TT, E])
            nc.vector.tensor_tensor(selm3, ca_a3, sel3, op=ALU.mult)
            cac_all = rt.tile([128, TT], F32, tag=f"cac")
            nc.vector.tensor_reduce(
                cac_all[:], selm3, mybir.AxisListType.X, ALU.add
            )
            ca_cols = [cac_all[:, j : j + 1] for j in range(TT)]
            oneminus = [om_all[:, j : j + 1] for j in range(TT)]

            # ---- expert gate/up + SwiGLU -> h [DF_E, CT] -------------
            hc = []
            for f in range(KF):
                psg = ps_gu.tile([128, CT], F32, tag="psgu")
                for k in range(KD):
                    nc.tensor.matmul(
                        psg[:],
                        gq[k][:, f * 128 : (f + 1) * 128],
                        xt[k][:],
                        start=(k == 0),
                        stop=(k == KD - 1),
                    )
                psu = ps_gu.tile([128, CT], F32, tag="psgu")
                for k in range(KD):
                    nc.tensor.matmul(
                        psu[:],
                        uq[k][:, f * 128 : (f + 1) * 128],
                        xt[k][:],
                        start=(k == 0),
                        stop=(k == KD - 1),
                    )
                sig = work.tile([128, CT], F32, tag="sig")
                nc.scalar.activation(sig[:], psg[:], ACTF.Sigmoid)
                sil = work.tile([128, CT], F32, tag="sil")
                nc.vector.tensor_mul(sil[:], sig[:], psg[:])
                h = hp.tile([128, CT], DT_MM, tag=f"h{f}")
                nc.vector.tensor_mul(h[:], sil[:], psu[:])
                hc.append(h)

            # ---- shared gate/up -> hs [FS, CT] -----------------------
            hsc = []
            for f in range(KS):
                psg = ps_gu.tile([128, CT], F32, tag="psgu")
                for k in range(KD):
                    nc.tensor.matmul(
                        psg[:],
                        wg[k][:, f * 128 : (f + 1) * 128],
                        xt[k][:],
                        start=(k == 0),
                        stop=(k == KD - 1),
                    )
                psu = ps_gu.tile([128, CT], F32, tag="psgu")
                for k in range(KD):
                    nc.tensor.matmul(
                        psu[:],
                        wu[k][:, f * 128 : (f + 1) * 128],
                        xt[k][:],
                        start=(k == 0),
                        stop=(k == KD - 1),
                    )
                sig = work.tile([128, CT], F32, tag="sig")
                nc.scalar.activation(sig[:], psg[:], ACTF.Sigmoid)
                sil = work.tile([128, CT], F32, tag="sil")
                nc.vector.tensor_mul(sil[:], sig[:], psg[:])
                hs = hp.tile([128, CT], DT_MM, tag=f"hs{f}")
                nc.vector.tensor_mul(hs[:], sil[:], psu[:])
                hsc.append(hs)

            # ---- down projections + combine + scatter to bounce ------
            rs_in = dram.tile([CT, D], F32, tag=f"rsin{c}")
            for j in range(TT):
                contrib = work.tile([128, D], F32, tag="contrib")
                for dd in range(ND):
                    pse = ps_dn.tile([128, 512], F32, tag="psd")
                    for k in range(KF):
                        nc.tensor.matmul(
                            pse[:],
                            hc[k][:, j * 128 : (j + 1) * 128],
                            dq[k][:, dd * 512 : (dd + 1) * 512],
                            start=(k == 0),
                            stop=(k == KF - 1),
                        )
                    pss = ps_dn.tile([128, 512], F32, tag="psd")
                    for k in range(KS):
                        nc.tensor.matmul(
                            pss[:],
                            hsc[k][:, j * 128 : (j + 1) * 128],
                            wd[k][:, dd * 512 : (dd + 1) * 512],
                            start=(k == 0),
                            stop=(k == KS - 1),
                        )
                    esc = work.tile([128, 512], F32, tag="esc")
                    nc.vector.tensor_scalar(
                        esc[:], pse[:], ca_cols[j], None, ALU.mult
                    )
                    nc.vector.scalar_tensor_tensor(
                        contrib[:, dd * 512 : (dd + 1) * 512],
                        pss[:],
                        oneminus[j],
                        esc[:],
                        ALU.mult,
                        ALU.add,
                    )
                # scalar-engine DGE queue: keeps stores off the sync
                # queues that feed the PE with x/weight loads
                nc.scalar.dma_start(
                    rs_in[j * 128 : (j + 1) * 128, :], contrib[:]
                )

            if c < CH - 1:
                rs_out = dram.tile([CT // NCORES, D], F32, tag=f"rsout{c}")
                nc.gpsimd.collective_compute(
                    "ReduceScatter",
                    ALU.add,
                    replica_groups=[list(range(NCORES))],
                    ins=[rs_in.opt()],
                    outs=[rs_out.opt()],
                )
                # gpsimd queue: this copy depends on the RS; keeping it
                # off the sync FIFO avoids head-of-line blocking
                nc.gpsimd.dma_start(OUT[c], rs_out[:])
            else:
                # split the final RS in half so the tail is one 1MB RS
                HALF = CT // 2
                for h in range(2):
                    rs_out_h = dram.tile(
                        [HALF // NCORES, D], F32, tag=f"rsout{c}_{h}"
                    )
                    nc.gpsimd.collective_compute(
                        "ReduceScatter",
                        ALU.add,
                        replica_groups=[list(range(NCORES))],
                        ins=[rs_in[h * HALF : (h + 1) * HALF, :].opt()],
                        outs=[rs_out_h.opt()],
                    )
                    nc.gpsimd.dma_start(
                        OUT[c, h * (HALF // NCORES) : (h + 1) * (HALF // NCORES)],
                        rs_out_h[:],
                    )

    nc.compile()
    return nc


def _prep_inputs(x, router_weight, sh_gate_w, sh_up_w, sh_down_w, gate_s,
                 up_s, down_s, alpha, gate_q, up_q, down_q):
    xf32 = np.ascontiguousarray(
        np.asarray(x, dtype=np.float32).reshape(T, D).T
    )
    xf = np.ascontiguousarray(xf32.astype(NP_MM))
    rwT = np.ascontiguousarray(np.asarray(router_weight, np.float32).T)
    in_maps = []
    for c in range(NCORES):
        gw = np.asarray(gate_q[c], np.float32) * np.asarray(
            gate_s[c], np.float32
        )[:, None]                                  # [DF_E, D]
        uw = np.asarray(up_q[c], np.float32) * np.asarray(
            up_s[c], np.float32
        )[:, None]                                  # [DF_E, D]
        dw = np.asarray(down_q[c], np.float32) * np.asarray(
            down_s[c], np.float32
        )[:, None]                                  # [D, DF_E]
        aux = np.zeros((128, 2 * E), np.float32)
        aux[:, 0:E] = np.asarray(alpha, np.float32)[None, :]
        aux[:, E + c] = 1.0
        in_maps.append(
            {
                "xT": xf,
                "xTf": xf32,
                "rwT": rwT,
                "gqT": np.ascontiguousarray(gw.T.astype(NP_MM)),
                "uqT": np.ascontiguousarray(uw.T.astype(NP_MM)),
                "dqT": np.ascontiguousarray(dw.T.astype(NP_MM)),
                "wgT": np.ascontiguousarray(
                    np.asarray(sh_gate_w[c * FS : (c + 1) * FS], np.float32)
                    .T.astype(NP_MM)
                ),
                "wuT": np.ascontiguousarray(
                    np.asarray(sh_up_w[c * FS : (c + 1) * FS], np.float32)
                    .T.astype(NP_MM)
                ),
                "wdT": np.ascontiguousarray(
                    np.asarray(sh_down_w[:, c * FS : (c + 1) * FS], np.float32)
                    .T.astype(NP_MM)
                ),
                "aux": aux,
            }
        )
    return in_maps


def assemble(outs):
    """Reassemble per-core OUT tensors [CH, 64, D] into [B, S, D]."""
    out = np.empty((T, D), np.float32)
    sh = CT // NCORES  # 64 rows per (chunk, rank)
    for r in range(NCORES):
        o = np.asarray(outs[r])
        for c in range(CH - 1):
            out[c * CT + r * sh : c * CT + (r + 1) * sh] = o[c]
        # last chunk was reduce-scattered in two halves of 256 tokens
        c = CH - 1
        hs = sh // 2  # 32 rows per (half, rank)
        for h in range(2):
            base = c * CT + h * (CT // 2) + r * hs
            out[base : base + hs] = o[c, h * hs : (h + 1) * hs]
    return out.reshape(B, S, D)


def kernel(x, router_weight, sh_gate_w, sh_up_w, sh_down_w, gate_s, up_s,
           down_s, alpha, gate_q, up_q, down_q, top_k, **run_kwargs):
    assert int(top_k) == 2, "kernel compiled for top_k=2"
    assert tuple(np.shape(x)) == (B, S, D)

    if "nc" not in _CACHE:
        _CACHE["nc"] = _build()
    nc = _CACHE["nc"]

    in_maps = _prep_inputs(
        x, router_weight, sh_gate_w, sh_up_w, sh_down_w, gate_s, up_s,
        down_s, alpha, gate_q, up_q, down_q,
    )
    res = run_bass_kernel_spmd(
        nc, in_maps, core_ids=list(range(NCORES)), **run_kwargs
    )
    _CACHE["last_results"] = res

    outs = [res.results[r]["OUT"] for r in range(NCORES)]
    return assemble(outs).astype(np.asarray(x).dtype)


# revision 19
# speedup vs baseline: 1.1978x; 1.1978x over previous
"""Routed quantized MoE eval kernel for 8 Trainium2 NeuronCores.

Strategy (expert-parallel, per sharding hint):
- Core c owns expert e=c: quantized expert weights are dequantized
  (scale-folded) + transposed on the host at shard-prep time; the
  matmuls, router, top-2 softmax, SwiGLU activations and combine all
  run on device.
- Shared MLP is sharded along DF_S: core c computes rows
  [256c, 256c+256) of the shared gate/up and the matching columns of
  the down projection, giving a partial shared output.
- Every core computes the full router (fp32 matmuls - top-2 selection
  is tie-sensitive), forms its own expert's combine column
  ca[:, e] * alpha[e] and (1 - sum_e ca*alpha), scales its expert
  output and shared partial, and the per-token sum across all 8 cores
  is taken by chunked ReduceScatter collectives that overlap compute.
- Big matmuls run as float32r (2 cyc/row, ~1.5e-4 rel err), router in
  true float32.

Output identity used:
  mixed = (1 - sum_e ca_e*alpha_e) * shared + sum_e ca_e*alpha_e * eo_e
where shared = sum over cores of shared partials, so each core's
contribution is (1-s)*shared_partial_c + ca_c*alpha_c*eo_c.
"""

import numpy as np
from contextlib import ExitStack

import concourse.bass as bass
import concourse.tile as tile
from concourse import bacc, mybir
from concourse.bass_utils import run_bass_kernel_spmd

NCORES = 8
B, S, D = 2, 1024, 1024
T = B * S                      # 2048 tokens
DF_E, DF_S, E = 512, 2048, 8
FS = DF_S // NCORES            # 256 shared-ffn rows per core
CH = 4                         # token chunks
CT = T // CH                   # 512 tokens per chunk
TT = CT // 128                 # 4 token tiles per chunk
KD = D // 128                  # 8 k-tiles over hidden dim
KF = DF_E // 128               # 4 k-tiles over expert ffn dim
KS = FS // 128                 # 2 k-tiles over shared ffn shard
ND = D // 512                  # 2 output column slices

import os

FR = mybir.dt.float32r
F16 = mybir.dt.float16
F32 = mybir.dt.float32
DT_MODE = os.environ.get("MOE_DT", "f32r")
DT_MM = {"f32r": FR, "f16": F16}[DT_MODE]
NP_MM = {"f32r": np.float32, "f16": np.float16}[DT_MODE]
ACTF = mybir.ActivationFunctionType
ALU = mybir.AluOpType

_CACHE = {}


def _build():
    nc = bacc.Bacc(
        "TRN2", target_bir_lowering=False, debug=False, num_devices=NCORES
    )

    xT = nc.dram_tensor("xT", [D, T], DT_MM, kind="ExternalInput").ap()
    xTf = nc.dram_tensor("xTf", [D, T], F32, kind="ExternalInput").ap()
    rwT = nc.dram_tensor("rwT", [D, E], F32, kind="ExternalInput").ap()
    gqT = nc.dram_tensor("gqT", [D, DF_E], DT_MM, kind="ExternalInput").ap()
    uqT = nc.dram_tensor("uqT", [D, DF_E], DT_MM, kind="ExternalInput").ap()
    dqT = nc.dram_tensor("dqT", [DF_E, D], DT_MM, kind="ExternalInput").ap()
    wgT = nc.dram_tensor("wgT", [D, FS], DT_MM, kind="ExternalInput").ap()
    wuT = nc.dram_tensor("wuT", [D, FS], DT_MM, kind="ExternalInput").ap()
    wdT = nc.dram_tensor("wdT", [FS, D], DT_MM, kind="ExternalInput").ap()
    # aux[:, 0:8] = alpha broadcast, aux[:, 8:16] = onehot(expert) broadcast
    aux = nc.dram_tensor("aux", [128, 2 * E], F32, kind="ExternalInput").ap()
    OUT = nc.dram_tensor(
        "OUT", [CH, CT // NCORES, D], F16, kind="ExternalOutput"
    ).ap()

    with ExitStack() as ctx:
        tc = ctx.enter_context(tile.TileContext(nc))
        wres = ctx.enter_context(tc.tile_pool(name="wres", bufs=1))
        xs = ctx.enter_context(tc.tile_pool(name="xs", bufs=2))
        xfp = ctx.enter_context(tc.tile_pool(name="xfp", bufs=1))
        hp = ctx.enter_context(tc.tile_pool(name="hp", bufs=2))
        work = ctx.enter_context(tc.tile_pool(name="work", bufs=2))
        rt = ctx.enter_context(tc.tile_pool(name="rt", bufs=2))
        ps_gu = ctx.enter_context(tc.tile_pool(name="ps_gu", bufs=3, space="PSUM"))
        ps_dn = ctx.enter_context(tc.tile_pool(name="ps_dn", bufs=3, space="PSUM"))
        ps_r = ctx.enter_context(tc.tile_pool(name="ps_r", bufs=2, space="PSUM"))
        dram = ctx.enter_context(tc.tile_pool(name="dram", bufs=1, space="DRAM"))

        # ---- resident weights ----------------------------------------
        def load_rows(src, rows, cols, name):
            tiles = []
            r = src.rearrange("(k p) n -> k p n", p=128)
            for k in range(rows // 128):
                t = wres.tile([128, cols], src.dtype, tag=f"{name}{k}")
                nc.sync.dma_start(t[:], r[k])
                tiles.append(t)
            return tiles

        # router weights + aux first (tiny, unblock router matmuls)
        from concourse.masks import make_identity

        ident = wres.tile([128, 128], F32, tag="ident")
        make_identity(nc, ident[:])
        rw = load_rows(rwT, D, E, "rw")
        aux_sb = wres.tile([128, 2 * E], F32, tag="aux")
        nc.sync.dma_start(aux_sb[:], aux[:])
        alpha_bc = aux_sb[:, 0:E]
        sel_bc = aux_sb[:, E : 2 * E]

        xTr = xT.rearrange("(k p) t -> k p t", p=128)
        xTfr = xTf.rearrange("(k p) t -> k p t", p=128)

        def load_x(c):
            xt, xf_t = [], []
            for k in range(KD):
                tf = xfp.tile([128, CT], F32, tag=f"xf{k}")
                nc.sync.dma_start(tf[:], xTfr[k, :, c * CT : (c + 1) * CT])
                xf_t.append(tf)
                t = xs.tile([128, CT], DT_MM, tag=f"xt{k}")
                nc.sync.dma_start(t[:], xTr[k, :, c * CT : (c + 1) * CT])
                xt.append(t)
            return xt, xf_t

        # tiny warmup ReduceScatter: absorbs the ~25us ncfw cold-start
        # before the first real collective needs to fire
        wu_in = dram.tile([NCORES, 128], F32, tag="wuin")
        wu_out = dram.tile([1, 128], F32, tag="wuout")
        nc.sync.dma_start(wu_in[:], xTf[0:NCORES, 0:128])
        nc.gpsimd.collective_compute(
            "ReduceScatter",
            ALU.add,
            replica_groups=[list(range(NCORES))],
            ins=[wu_in.opt()],
            outs=[wu_out.opt()],
        )
        x_pre = load_x(0)
        gq = load_rows(gqT, D, DF_E, "gq")
        uq = load_rows(uqT, D, DF_E, "uq")
        dq = load_rows(dqT, DF_E, D, "dq")
        wg = load_rows(wgT, D, FS, "wg")
        wu = load_rows(wuT, D, FS, "wu")
        wd = load_rows(wdT, FS, D, "wd")

        for c in range(CH):
            xt, xf_t = x_pre
            if c + 1 < CH:
                x_pre = load_x(c + 1)

            # ---- router + combine weights ----------------------------
            ps_lt = ps_r.tile([E, CT], F32, tag="psr")
            for k in range(KD):
                nc.tensor.matmul(
                    ps_lt[:],
                    rw[k][:],
                    xf_t[k][:],
                    start=(k == 0),
                    stop=(k == KD - 1),
                )
            Lt = rt.tile([E, CT], F32, tag="Lt")
            nc.vector.tensor_copy(Lt[:], ps_lt[:])
            # transpose the 4 token-tile blocks into one [128, TT*E] tile
            ps_l = ps_r.tile([128, TT * E], F32, tag="psr")
            for j in range(TT):
                nc.tensor.transpose(
                    ps_l[:, j * E : (j + 1) * E],
                    Lt[:, j * 128 : (j + 1) * 128],
                    ident[0:E, 0:E],
                )
            L = rt.tile([128, TT * E], F32, tag="L")
            nc.vector.tensor_copy(L[:], ps_l[:])
            L3 = L[:].rearrange("p (j e) -> p j e", e=E)

            def bc(t):  # [128, TT] -> [128, TT, E] free-axis broadcast
                return t[:, :, None].broadcast_to([128, TT, E])

            m1 = rt.tile([128, TT], F32, tag="m1")
            nc.vector.tensor_reduce(m1[:], L3, mybir.AxisListType.X, ALU.max)
            mask1 = rt.tile([128, TT * E], F32, tag="mask1")
            mask1_3 = mask1[:].rearrange("p (j e) -> p j e", e=E)
            nc.vector.tensor_tensor(mask1_3, L3, bc(m1), op=ALU.is_ge)
            L2 = rt.tile([128, TT * E], F32, tag="L2")
            nc.vector.scalar_tensor_tensor(
                L2[:], mask1[:], -1e30, L[:], ALU.mult, ALU.add
            )
            L2_3 = L2[:].rearrange("p (j e) -> p j e", e=E)
            m2 = rt.tile([128, TT], F32, tag="m2")
            nc.vector.tensor_reduce(m2[:], L2_3, mybir.AxisListType.X, ALU.max)
            mask2 = rt.tile([128, TT * E], F32, tag="mask2")
            mask2_3 = mask2[:].rearrange("p (j e) -> p j e", e=E)
            nc.vector.tensor_tensor(mask2_3, L2_3, bc(m2), op=ALU.is_ge)
            # softmax over {m1, m2}: w1 = sigmoid(m1 - m2), w2 = 1 - w1
            dlt = rt.tile([128, TT], F32, tag="dlt")
            nc.vector.tensor_sub(dlt[:], m1[:], m2[:])
            w1 = rt.tile([128, TT], F32, tag="w1")
            nc.scalar.activation(w1[:], dlt[:], ACTF.Sigmoid)
            w2 = rt.tile([128, TT], F32, tag="w2")
            nc.vector.tensor_scalar(w2[:], w1[:], -1.0, 1.0, ALU.mult, ALU.add)
            caw = rt.tile([128, TT * E], F32, tag="caw")
            caw3 = caw[:].rearrange("p (j e) -> p j e", e=E)
            nc.vector.tensor_tensor(caw3, mask2_3, bc(w2), op=ALU.mult)
            t1 = rt.tile([128, TT * E], F32, tag="t1")
            t1_3 = t1[:].rearrange("p (j e) -> p j e", e=E)
            nc.vector.tensor_tensor(t1_3, mask1_3, bc(w1), op=ALU.mult)
            nc.vector.tensor_add(caw[:], caw[:], t1[:])
            # scale by alpha (broadcast over token-tiles) and reduce
            ca_a = rt.tile([128, TT * E], F32, tag="ca_a")
            ca_a3 = ca_a[:].rearrange("p (j e) -> p j e", e=E)
            alpha3 = alpha_bc[:, None, :].broadcast_to([128, TT, E])
            nc.vector.tensor_tensor(ca_a3, caw3, alpha3, op=ALU.mult)
            s = rt.tile([128, TT], F32, tag="s")
            nc.vector.tensor_reduce(s[:], ca_a3, mybir.AxisListType.X, ALU.add)
            om_all = rt.tile([128, TT], F32, tag=f"om")
            nc.vector.tensor_scalar(om_all[:], s[:], -1.0, 1.0, ALU.mult, ALU.add)
            selm = rt.tile([128, TT * E], F32, tag="selm")
            selm3 = selm[:].rearrange("p (j e) -> p j e", e=E)
            sel3 = sel_bc[:, None, :].broadcast_to([128, TT, E])
            nc.vector.tensor_tensor(selm3, ca_a3, sel3, op=ALU.mult)
            cac_all = rt.tile([128, TT], F32, tag=f"cac")
            nc.vector.tensor_reduce(
                cac_all[:], selm3, mybir.AxisListType.X, ALU.add
            )
            ca_cols = [cac_all[:, j : j + 1] for j in range(TT)]
            oneminus = [om_all[:, j : j + 1] for j in range(TT)]

            # ---- expert gate/up + SwiGLU -> h [DF_E, CT] -------------
            hc = []
            for f in range(KF):
                psg = ps_gu.tile([128, CT], F32, tag="psgu")
                for k in range(KD):
                    nc.tensor.matmul(
                        psg[:],
                        gq[k][:, f * 128 : (f + 1) * 128],
                        xt[k][:],
                        start=(k == 0),
                        stop=(k == KD - 1),
                    )
                psu = ps_gu.tile([128, CT], F32, tag="psgu")
                for k in range(KD):
                    nc.tensor.matmul(
                        psu[:],
                        uq[k][:, f * 128 : (f + 1) * 128],
                        xt[k][:],
                        start=(k == 0),
                        stop=(k == KD - 1),
                    )
                sig = work.tile([128, CT], F32, tag="sig")
                nc.scalar.activation(sig[:], psg[:], ACTF.Sigmoid)
                sil = work.tile([128, CT], F32, tag="sil")
                nc.vector.tensor_mul(sil[:], sig[:], psg[:])
                h = hp.tile([128, CT], DT_MM, tag=f"h{f}")
                nc.vector.tensor_mul(h[:], sil[:], psu[:])
                hc.append(h)

            # ---- shared gate/up -> hs [FS, CT] -----------------------
            hsc = []
            for f in range(KS):
                psg = ps_gu.tile([128, CT], F32, tag="psgu")
                for k in range(KD):
                    nc.tensor.matmul(
                        psg[:],
                        wg[k][:, f * 128 : (f + 1) * 128],
                        xt[k][:],
                        start=(k == 0),
                        stop=(k == KD - 1),
                    )
                psu = ps_gu.tile([128, CT], F32, tag="psgu")
                for k in range(KD):
                    nc.tensor.matmul(
                        psu[:],
                        wu[k][:, f * 128 : (f + 1) * 128],
                        xt[k][:],
                        start=(k == 0),
                        stop=(k == KD - 1),
                    )
                sig = work.tile([128, CT], F32, tag="sig")
                nc.scalar.activation(sig[:], psg[:], ACTF.Sigmoid)
                sil = work.tile([128, CT], F32, tag="sil")
                nc.vector.tensor_mul(sil[:], sig[:], psg[:])
                hs = hp.tile([128, CT], DT_MM, tag=f"hs{f}")
                nc.vector.tensor_mul(hs[:], sil[:], psu[:])
                hsc.append(hs)

            # ---- down projections + combine + scatter to bounce ------
            rs_in = dram.tile([CT, D], F16, tag=f"rsin{c}")
            for j in range(TT):
                contrib = work.tile([128, D], F16, tag="contrib")
                for dd in range(ND):
                    pse = ps_dn.tile([128, 512], F32, tag="psd")
                    for k in range(KF):
                        nc.tensor.matmul(
                            pse[:],
                            hc[k][:, j * 128 : (j + 1) * 128],
                            dq[k][:, dd * 512 : (dd + 1) * 512],
                            start=(k == 0),
                            stop=(k == KF - 1),
                        )
                    pss = ps_dn.tile([128, 512], F32, tag="psd")
                    for k in range(KS):
                        nc.tensor.matmul(
                            pss[:],
                            hsc[k][:, j * 128 : (j + 1) * 128],
                            wd[k][:, dd * 512 : (dd + 1) * 512],
                            start=(k == 0),
                            stop=(k == KS - 1),
                        )
                    esc = work.tile([128, 512], F32, tag="esc")
                    nc.vector.tensor_scalar(
                        esc[:], pse[:], ca_cols[j], None, ALU.mult
                    )
                    nc.vector.scalar_tensor_tensor(
                        contrib[:, dd * 512 : (dd + 1) * 512],
                        pss[:],
                        oneminus[j],
                        esc[:],
                        ALU.mult,
                        ALU.add,
                    )
                # scalar-engine DGE queue: keeps stores off the sync
                # queues that feed the PE with x/weight loads
                nc.scalar.dma_start(
                    rs_in[j * 128 : (j + 1) * 128, :], contrib[:]
                )

            if c < CH - 1:
                rs_out = dram.tile([CT // NCORES, D], F16, tag=f"rsout{c}")
                nc.gpsimd.collective_compute(
                    "ReduceScatter",
                    ALU.add,
                    replica_groups=[list(range(NCORES))],
                    ins=[rs_in.opt()],
                    outs=[rs_out.opt()],
                )
                # gpsimd queue: this copy depends on the RS; keeping it
                # off the sync FIFO avoids head-of-line blocking
                nc.gpsimd.dma_start(OUT[c], rs_out[:])
            else:
                # split the final RS in half so the tail is one 1MB RS
                HALF = CT // 2
                for h in range(2):
                    rs_out_h = dram.tile(
                        [HALF // NCORES, D], F16, tag=f"rsout{c}_{h}"
                    )
                    nc.gpsimd.collective_compute(
                        "ReduceScatter",
                        ALU.add,
                        replica_groups=[list(range(NCORES))],
                        ins=[rs_in[h * HALF : (h + 1) * HALF, :].opt()],
                        outs=[rs_out_h.opt()],
                    )
                    nc.gpsimd.dma_start(
                        OUT[c, h * (HALF // NCORES) : (h + 1) * (HALF // NCORES)],
                        rs_out_h[:],
                    )

    nc.compile()
    return nc


def _prep_inputs(x, router_weight, sh_gate_w, sh_up_w, sh_down_w, gate_s,
                 up_s, down_s, alpha, gate_q, up_q, down_q):
    xf32 = np.ascontiguousarray(
        np.asarray(x, dtype=np.float32).reshape(T, D).T
    )
    xf = np.ascontiguousarray(xf32.astype(NP_MM))
    rwT = np.ascontiguousarray(np.asarray(router_weight, np.float32).T)
    in_maps = []
    for c in range(NCORES):
        gw = np.asarray(gate_q[c], np.float32) * np.asarray(
            gate_s[c], np.float32
        )[:, None]                                  # [DF_E, D]
        uw = np.asarray(up_q[c], np.float32) * np.asarray(
            up_s[c], np.float32
        )[:, None]                                  # [DF_E, D]
        dw = np.asarray(down_q[c], np.float32) * np.asarray(
            down_s[c], np.float32
        )[:, None]                                  # [D, DF_E]
        aux = np.zeros((128, 2 * E), np.float32)
        aux[:, 0:E] = np.asarray(alpha, np.float32)[None, :]
        aux[:, E + c] = 1.0
        in_maps.append(
            {
                "xT": xf,
                "xTf": xf32,
                "rwT": rwT,
                "gqT": np.ascontiguousarray(gw.T.astype(NP_MM)),
                "uqT": np.ascontiguousarray(uw.T.astype(NP_MM)),
                "dqT": np.ascontiguousarray(dw.T.astype(NP_MM)),
                "wgT": np.ascontiguousarray(
                    np.asarray(sh_gate_w[c * FS : (c + 1) * FS], np.float32)
                    .T.astype(NP_MM)
                ),
                "wuT": np.ascontiguousarray(
                    np.asarray(sh_up_w[c * FS : (c + 1) * FS], np.float32)
                    .T.astype(NP_MM)
                ),
                "wdT": np.ascontiguousarray(
                    np.asarray(sh_down_w[:, c * FS : (c + 1) * FS], np.float32)
                    .T.astype(NP_MM)
                ),
                "aux": aux,
            }
        )
    return in_maps


def assemble(outs):
    """Reassemble per-core OUT tensors [CH, 64, D] into [B, S, D]."""
    out = np.empty((T, D), np.float32)
    sh = CT // NCORES  # 64 rows per (chunk, rank)
    for r in range(NCORES):
        o = np.asarray(outs[r])
        for c in range(CH - 1):
            out[c * CT + r * sh : c * CT + (r + 1) * sh] = o[c]
        # last chunk was reduce-scattered in two halves of 256 tokens
        c = CH - 1
        hs = sh // 2  # 32 rows per (half, rank)
        for h in range(2):
            base = c * CT + h * (CT // 2) + r * hs
            out[base : base + hs] = o[c, h * hs : (h + 1) * hs]
    return out.reshape(B, S, D)


def kernel(x, router_weight, sh_gate_w, sh_up_w, sh_down_w, gate_s, up_s,
           down_s, alpha, gate_q, up_q, down_q, top_k, **run_kwargs):
    assert int(top_k) == 2, "kernel compiled for top_k=2"
    assert tuple(np.shape(x)) == (B, S, D)

    if "nc" not in _CACHE:
        _CACHE["nc"] = _build()
    nc = _CACHE["nc"]

    in_maps = _prep_inputs(
        x, router_weight, sh_gate_w, sh_up_w, sh_down_w, gate_s, up_s,
        down_s, alpha, gate_q, up_q, down_q,
    )
    res = run_bass_kernel_spmd(
        nc, in_maps, core_ids=list(range(NCORES)), **run_kwargs
    )
    _CACHE["last_results"] = res

    outs = [res.results[r]["OUT"] for r in range(NCORES)]
    return assemble(outs).astype(np.asarray(x).dtype)
